# revision 7
# baseline (speedup 1.0000x reference)
"""Trainium2 Bass kernel for teacher-forced LSTM decoder with Bahdanau attention.

Problem: B=32, S=64, T=64, H=512, V=32000.
Sharding: data-parallel over batch across 8 cores (4 rows each). Each core runs
the full recurrence + attention for its batch rows and a full-vocab output
projection + log_softmax locally (no collectives).

Device-side structure per core:
  init:   scale i/f/o gate rows by 0.5 (sigmoid(x) = 0.5*tanh(x/2)+0.5),
          tk = enc @ Wk.T + (bq+bk)  in (b,s)-partition layout,
          P = enc @ Wih_ctx.T        (context folded into gates: Wih_ctx@ctx ==
                                      P.T @ attn, contracted on the PE),
          G0 = emb_seq @ Wih_emb.T + (bih+bhh) -> DRAM scratch (per-step rows)
  step t: tq = h @ Wq.T; replicate per (b,s)-group via E-matrix matmul;
          arg = tq_rep + tk; tanh on ACT; scores = sum_h(tanh*Ws) fused on DVE;
          softmax without max-subtraction (scores are small); attention applied
          through a block-diagonal exp matrix as matmul lhsT; gates =
          Whh@h + P.T@attn + G0[t]; tanh-form LSTM pointwise; h2 transposed
          back via PE transpose into the lhsT buffer for step t+1.
  proj:   logits = h2_all @ Wout.T + bout (Wout streamed once, bf16),
          online sumexp via ACT accum, logits cached bf16 in SBUF,
          log_probs = logits - ln(sumexp), streamed to DRAM.
"""
import sys

for _p in ("/opt/trn_rl_repo",):
    if _p not in sys.path:
        sys.path.insert(0, _p)

import numpy as np
import ml_dtypes

import concourse.bass as bass
import concourse.mybir as mybir
import concourse.tile as tile
from concourse import bacc
from concourse.bass_utils import run_bass_kernel_spmd

F32 = mybir.dt.float32
F32R = mybir.dt.float32r
BF16 = mybir.dt.bfloat16
ADD = mybir.AluOpType.add
MULT = mybir.AluOpType.mult
TANH = mybir.ActivationFunctionType.Tanh
EXP = mybir.ActivationFunctionType.Exp
LN = mybir.ActivationFunctionType.Ln

B, S, T, H, V = 32, 64, 64, 512, 32000
BL = 4          # batch rows per core
NCORES = 8
NT = 63         # vocab n-tiles of 512 (62*512 + 256)
VP = NT * 512   # padded vocab 32256
G4 = 4 * H      # 2048 gate dims


def _emit_init(nc, tc, tk2, P2, h2aT, cst, ones1, bd,
               encT_in, embT_in, WkT_in, WihcT_in, WiheT_in,
               bq_in, bk_in, bih_in, bhh_in, G0_dram):
    with (
        tc.tile_pool(name="init_sb", bufs=1) as isb,
        tc.tile_pool(name="init_ps", bufs=1, space="PSUM") as ips,
    ):
        encT = isb.tile([128, 4, 2, 128], F32R, name="encT")
        nc.sync.dma_start(encT[:], encT_in[:])
        embT = isb.tile([128, 4, 2, 128], F32R, name="embT")
        nc.sync.dma_start(embT[:], embT_in[:])
        WkT = isb.tile([128, 4, H], F32R, name="WkT")
        nc.sync.dma_start(WkT[:], WkT_in[:])
        WihcT = isb.tile([128, 4, G4], F32R, name="WihcT")
        nc.sync.dma_start(WihcT[:], WihcT_in[:])
        WiheT = isb.tile([128, 4, G4], F32R, name="WiheT")
        nc.sync.dma_start(WiheT[:], WiheT_in[:])
        nc.vector.tensor_scalar_mul(WihcT[:, :, 0:1024], WihcT[:, :, 0:1024], 0.5)
        nc.vector.tensor_scalar_mul(WihcT[:, :, 1536:2048], WihcT[:, :, 1536:2048], 0.5)
        nc.vector.tensor_scalar_mul(WiheT[:, :, 0:1024], WiheT[:, :, 0:1024], 0.5)
        nc.vector.tensor_scalar_mul(WiheT[:, :, 1536:2048], WiheT[:, :, 1536:2048], 0.5)

        bq = isb.tile([1, H], F32, name="bq")
        nc.sync.dma_start(bq[:], bq_in[:])
        bk = isb.tile([1, H], F32, name="bk")
        nc.sync.dma_start(bk[:], bk_in[:])
        bih = isb.tile([1, G4], F32, name="bih")
        nc.sync.dma_start(bih[:], bih_in[:])
        bhh = isb.tile([1, G4], F32, name="bhh")
        nc.sync.dma_start(bhh[:], bhh_in[:])
        bqk = isb.tile([1, H], F32R, name="bqk")
        nc.vector.tensor_tensor(bqk[:], bq[:], bk[:], ADD)
        bgate = isb.tile([1, G4], F32R, name="bgate")
        nc.vector.tensor_tensor(bgate[:], bih[:], bhh[:], ADD)
        nc.vector.tensor_scalar_mul(bgate[:, 0:1024], bgate[:, 0:1024], 0.5)
        nc.vector.tensor_scalar_mul(bgate[:, 1536:2048], bgate[:, 1536:2048], 0.5)

        # replicate gate bias across partitions via ones-matmul
        ps_b = ips.tile([128, G4], F32, name="ps_b", tag="initG")
        for n in range(4):
            nc.tensor.matmul(ps_b[:, 512 * n:512 * n + 512], ones1[:],
                             bgate[:, 512 * n:512 * n + 512], start=True, stop=True)
        brep = isb.tile([128, G4], F32, name="brep")
        nc.vector.tensor_copy(brep[:], ps_b[:])
        ps_bq = ips.tile([128, H], F32, name="ps_bq", tag="initS", bufs=2)
        nc.tensor.matmul(ps_bq[:], ones1[:], bqk[:], start=True, stop=True)
        bqrep = isb.tile([128, H], F32, name="bqrep")
        nc.vector.tensor_copy(bqrep[:], ps_bq[:])

        # tk2[:, h, :] = sum_k encT[:,k,h,:].T @ WkT[:,k,:] + (bq+bk)
        for h in range(2):
            ps_k = ips.tile([128, H], F32, name=f"ps_k{h}", tag="initS", bufs=2)
            for k in range(4):
                nc.tensor.matmul(ps_k[:], encT[:, k, h, :], WkT[:, k, :],
                                 start=(k == 0), stop=(k == 3))
            nc.vector.tensor_tensor(tk2[:, h, :], ps_k[:], bqrep[:], ADD)

        # P2[:, h, :] = sum_k encT[:,k,h,:].T @ WihcT[:,k,:]
        for h in range(2):
            ps_p = ips.tile([128, G4], F32, name=f"ps_p{h}", tag="initG")
            for n in range(4):
                for k in range(4):
                    nc.tensor.matmul(ps_p[:, 512 * n:512 * n + 512],
                                     encT[:, k, h, :],
                                     WihcT[:, k, 512 * n:512 * n + 512],
                                     start=(k == 0), stop=(k == 3))
            nc.vector.tensor_copy(P2[:, h, :], ps_p[:])

        # G0 rows (t,b) -> DRAM scratch
        for m in range(2):
            ps_g0 = ips.tile([128, G4], F32, name=f"ps_g0{m}", tag="initG")
            for n in range(4):
                for k in range(4):
                    nc.tensor.matmul(ps_g0[:, 512 * n:512 * n + 512],
                                     embT[:, k, m, :],
                                     WiheT[:, k, 512 * n:512 * n + 512],
                                     start=(k == 0), stop=(k == 3))
            g0sb = isb.tile([128, G4], F32, name=f"g0sb{m}", tag="g0sb", bufs=2)
            nc.vector.tensor_tensor(g0sb[:], ps_g0[:], brep[:], ADD)
            nc.sync.dma_start(G0_dram[128 * m:128 * m + 128, :], g0sb[:])


def _emit_step(nc, t, rsb, rps, gsb, h2aT, cst, WqT, WhhT, tk2, P2, Wsrep,
               E4, E4T, id4, bd, attn_all, half4, G0_dram):
    hc = 4 * t
    ho = 4 * (t + 1)

    g0t = gsb.tile([4, G4], F32, name=f"g0t{t}", tag="g0t")
    nc.sync.dma_start(g0t[:], G0_dram[4 * t:4 * t + 4, :])

    # tq = h @ Wq.T  [4, H]
    ps_tq = rps.tile([4, H], F32, name=f"ps_tq{t}", tag="psA", bufs=2)
    for k in range(4):
        nc.tensor.matmul(ps_tq[:], h2aT[:, k, hc:hc + 4], WqT[:, k, :],
                         start=(k == 0), stop=(k == 3))
    tq_sb = rsb.tile([4, H], F32R, name=f"tq{t}", tag="tq")
    nc.vector.tensor_copy(tq_sb[:], ps_tq[:])

    # replicate per 32-group: ps_rep[p,:] = tq[p//32,:]
    ps_rep = rps.tile([128, H], F32, name=f"ps_rep{t}", tag="psA", bufs=2)
    nc.tensor.matmul(ps_rep[:], E4[:], tq_sb[:], start=True, stop=True)

    # attention scores per s-half
    scores = rsb.tile([128, 2], F32, name=f"sc{t}", tag="sc")
    for h in range(2):
        argt = rsb.tile([128, H], F32, name=f"arg{t}_{h}", tag="arg")
        nc.vector.tensor_tensor(argt[:], ps_rep[:], tk2[:, h, :], ADD)
        tht = rsb.tile([128, H], F32, name=f"th{t}_{h}", tag="th")
        nc.scalar.activation(tht[:], argt[:], TANH)
        scr = rsb.tile([128, H], F32, name=f"scr{t}_{h}", tag="scr")
        nc.vector.scalar_tensor_tensor(scr[:], tht[:], 1.0, Wsrep[:], MULT, MULT,
                                       accum_out=scores[:, h:h + 1])

    expt = rsb.tile([128, 2], F32, name=f"expt{t}", tag="expt")
    nc.scalar.activation(expt[:], scores[:], EXP)
    exptR = rsb.tile([128, 2], F32R, name=f"exptR{t}", tag="exptR")
    nc.vector.tensor_copy(exptR[:], expt[:])

    # sumexp per b: E4T.T @ exptR -> [4, 2]; reduce X -> [4, 1]
    ps_se = rps.tile([4, 2], F32, name=f"ps_se{t}", tag="psS", bufs=2)
    nc.tensor.matmul(ps_se[:], E4T[:], exptR[:], start=True, stop=True)
    se_sb = rsb.tile([4, 1], F32, name=f"se{t}", tag="se")
    nc.vector.tensor_reduce(se_sb[:], ps_se[:], axis=mybir.AxisListType.X,
                            op=mybir.AluOpType.add)
    rc = rsb.tile([4, 1], F32, name=f"rc{t}", tag="rc")
    nc.vector.reciprocal(rc[:], se_sb[:])
    rcr = rsb.tile([4, 2], F32R, name=f"rcr{t}", tag="rcr")
    nc.vector.tensor_copy(rcr[:], rc[:, 0:1].broadcast_to([4, 2]))

    # replicate recip per 32-group (N=2: single-column matmul fails ISA check)
    ps_rr = rps.tile([128, 2], F32, name=f"ps_rr{t}", tag="psS", bufs=2)
    nc.tensor.matmul(ps_rr[:], E4[:], rcr[:], start=True, stop=True)

    # normalize exp in place (per 32-group scalar)
    for b in range(4):
        sl = slice(32 * b, 32 * b + 32)
        nc.vector.tensor_tensor(expt[sl, :], expt[sl, :],
                                ps_rr[sl, 0:1].broadcast_to([32, 2]), MULT)
    nc.vector.tensor_copy(attn_all[:, t, :], expt[:])

    # block-diagonal attention weights (f32r for the PE)
    for h in range(2):
        for b in range(4):
            sl = slice(32 * b, 32 * b + 32)
            nc.vector.tensor_copy(bd[sl, h, b:b + 1], expt[sl, h:h + 1])

    # gates = Whh@h + P.T@attn  [4, G4]
    ps_g = rps.tile([4, G4], F32, name=f"ps_g{t}", tag="psG", bufs=1)
    for n in range(4):
        nsl = slice(512 * n, 512 * n + 512)
        for k in range(4):
            nc.tensor.matmul(ps_g[:, nsl], h2aT[:, k, hc:hc + 4], WhhT[:, k, nsl],
                             start=(k == 0), stop=False)
        for h in range(2):
            nc.tensor.matmul(ps_g[:, nsl], bd[:, h, :], P2[:, h, nsl],
                             start=False, stop=(h == 1))

    gates = rsb.tile([4, G4], F32, name=f"gates{t}", tag="gates")
    nc.vector.tensor_tensor(gates[:], ps_g[:], g0t[:], ADD)

    th = rsb.tile([4, G4], F32, name=f"thg{t}", tag="thg")
    nc.scalar.activation(th[:], gates[:], TANH)

    # sigmoid(x) = 0.5*tanh(x/2)+0.5 (weights pre-halved)
    s_if = rsb.tile([4, 1024], F32, name=f"sif{t}", tag="sif")
    nc.vector.scalar_tensor_tensor(s_if[:], th[:, 0:1024], 0.5,
                                   half4[:, 0:1].broadcast_to([4, 1024]),
                                   MULT, ADD)
    s_o = rsb.tile([4, H], F32, name=f"so{t}", tag="so")
    nc.vector.scalar_tensor_tensor(s_o[:], th[:, 1536:2048], 0.5,
                                   half4[:, 0:1].broadcast_to([4, H]),
                                   MULT, ADD)
    m1 = rsb.tile([4, H], F32, name=f"m1{t}", tag="m1")
    nc.vector.tensor_tensor(m1[:], s_if[:, 512:1024], cst[:], MULT)
    m2 = rsb.tile([4, H], F32, name=f"m2{t}", tag="m2")
    nc.vector.tensor_tensor(m2[:], s_if[:, 0:512], th[:, 1024:1536], MULT)
    nc.vector.tensor_tensor(cst[:], m1[:], m2[:], ADD)
    thc2 = rsb.tile([4, H], F32, name=f"thc2{t}", tag="thc2")
    nc.scalar.activation(thc2[:], cst[:], TANH)
    h2 = rsb.tile([4, H], F32, name=f"h2{t}", tag="h2")
    nc.vector.tensor_tensor(h2[:], s_o[:], thc2[:], MULT)

    # transpose h2 into h2aT[:, :, ho:ho+4]
    ps_hT = rps.tile([128, 4, 4], F32, name=f"ps_hT{t}", tag="psA", bufs=2)
    for k in range(4):
        nc.tensor.transpose(ps_hT[:, k, :], h2[:, 128 * k:128 * k + 128], id4[:])
    for k in range(4):
        nc.vector.tensor_copy(h2aT[:, k, ho:ho + 4], ps_hT[:, k, :])


def _emit_projection(nc, tc, h2aT, ones1, WoutT_in, bout_in, logp_out):
    with (
        tc.tile_pool(name="proj_sb", bufs=1) as qsb,
        tc.tile_pool(name="proj_io", bufs=3) as qio,
        tc.tile_pool(name="proj_ps", bufs=2, space="PSUM") as qps,
    ):
        h2bT = qsb.tile([128, 4, 256], BF16, name="h2bT")
        for k in range(4):
            nc.vector.tensor_copy(h2bT[:, k, :], h2aT[:, k, 4:4 + 256])

        for m in range(2):
            lgb = qsb.tile([128, NT, 512], BF16, name=f"lgb{m}", tag="lgb", bufs=2)
            tsums = qsb.tile([128, NT], F32, name=f"tsums{m}", tag="tsums", bufs=2)
            for n in range(NT):
                nj = 512 if n < NT - 1 else 256
                wo = qio.tile([128, 4, 512], BF16, name=f"wo{m}_{n}", tag="wo")
                nc.sync.dma_start(wo[:], WoutT_in[n, :, :, :])
                bo = qio.tile([1, 512], F32R, name=f"bo{m}_{n}", tag="bo")
                nc.sync.dma_start(bo[:, 0:nj], bout_in[0:1, 512 * n:512 * n + nj])
                ps_l = qps.tile([128, 512], F32, name=f"ps_l{m}_{n}", tag="ps_l")
                for k in range(4):
                    nc.tensor.matmul(ps_l[:, 0:nj], h2bT[:, k, 128 * m:128 * m + 128],
                                     wo[:, k, 0:nj], start=(k == 0), stop=False)
                nc.tensor.matmul(ps_l[:, 0:nj], ones1[:], bo[:, 0:nj],
                                 start=False, stop=True)
                scr = qio.tile([128, 512], F32, name=f"pscr{m}_{n}", tag="pscr")
                nc.scalar.activation(scr[:, 0:nj], ps_l[:, 0:nj], EXP,
                                     accum_out=tsums[:, n:n + 1])
                nc.any.tensor_copy(lgb[:, n, 0:nj], ps_l[:, 0:nj])

            ssum = qsb.tile([128, 1], F32, name=f"ssum{m}", tag="ssum", bufs=2)
            nc.vector.tensor_reduce(ssum[:], tsums[:], axis=mybir.AxisListType.X,
                                    op=mybir.AluOpType.add)
            lse = qsb.tile([128, 1], F32, name=f"lse{m}", tag="lse", bufs=2)
            nc.scalar.activation(lse[:], ssum[:], LN)

            for n in range(NT):
                nj = 512 if n < NT - 1 else 256
                ot = qio.tile([128, 512], F32, name=f"ot{m}_{n}", tag="ot")
                nc.vector.tensor_scalar_sub(ot[:, 0:nj], lgb[:, n, 0:nj], lse[:, 0:1])
                nc.sync.dma_start(logp_out[m, :, 512 * n:512 * n + nj], ot[:, 0:nj])


def build_module():
    nc = bacc.Bacc("TRN2", target_bir_lowering=False, debug=False)

    def din(name, shape, dt=F32R):
        return nc.dram_tensor(name, shape, dt, kind="ExternalInput")

    h0T_in = din("h0T_in", [128, 4, BL])
    c0_in = din("c0_in", [BL, H], F32)
    encT_in = din("encT_in", [128, 4, 2, 128])
    embT_in = din("embT_in", [128, 4, 2, 128])
    WqT_in = din("WqT_in", [128, 4, H])
    WkT_in = din("WkT_in", [128, 4, H])
    WhhT_in = din("WhhT_in", [128, 4, G4])
    WihcT_in = din("WihcT_in", [128, 4, G4])
    WiheT_in = din("WiheT_in", [128, 4, G4])
    bq_in = din("bq_in", [1, H], F32)
    bk_in = din("bk_in", [1, H], F32)
    bih_in = din("bih_in", [1, G4], F32)
    bhh_in = din("bhh_in", [1, G4], F32)
    Wsrep_in = din("Wsrep_in", [128, H], F32)
    E4_in = din("E4_in", [4, 128])
    E4T_in = din("E4T_in", [128, 4])
    ones1_in = din("ones1_in", [1, 128])
    id4_in = din("id4_in", [4, 4], F32)
    zeros8_in = din("zeros8_in", [128, 2, 4])
    WoutT_in = din("WoutT_in", [NT, 128, 4, 512], BF16)
    bout_in = din("bout_in", [1, VP])

    logp_out = nc.dram_tensor("logp_out", [2, 128, VP], F32, kind="ExternalOutput")
    attn_out = nc.dram_tensor("attn_out", [128, T, 2], F32, kind="ExternalOutput")
    hT_out = nc.dram_tensor("hT_out", [128, 4, BL], F32R, kind="ExternalOutput")
    cT_out = nc.dram_tensor("cT_out", [BL, H], F32, kind="ExternalOutput")

    G0_dram = nc.dram_tensor("G0_scratch", [T * BL, G4], F32)

    with tile.TileContext(nc) as tc:
        with tc.tile_pool(name="persist", bufs=1) as pp:
            h2aT = pp.tile([128, 4, 4 * (T + 1)], F32R, name="h2aT")
            cst = pp.tile([BL, H], F32, name="cst")
            E4 = pp.tile([4, 128], F32R, name="E4")
            E4T = pp.tile([128, 4], F32R, name="E4T")
            ones1 = pp.tile([1, 128], F32R, name="ones1")
            id4 = pp.tile([4, 4], F32, name="id4")
            bd = pp.tile([128, 2, 4], F32R, name="bd")
            attn_all = pp.tile([128, T, 2], F32, name="attn_all")
            half4 = pp.tile([4, 1], F32, name="half4")

            nc.sync.dma_start(h2aT[:, :, 0:4], h0T_in[:])
            nc.sync.dma_start(cst[:], c0_in[:])
            nc.sync.dma_start(E4[:], E4_in[:])
            nc.sync.dma_start(E4T[:], E4T_in[:])
            nc.sync.dma_start(ones1[:], ones1_in[:])
            nc.sync.dma_start(id4[:], id4_in[:])
            nc.sync.dma_start(bd[:], zeros8_in[:])
            nc.vector.memset(half4[:], 0.5)

            with tc.tile_pool(name="recw", bufs=1) as rw:
                WqT = rw.tile([128, 4, H], F32R, name="WqT")
                WhhT = rw.tile([128, 4, G4], F32R, name="WhhT")
                tk2 = rw.tile([128, 2, H], F32, name="tk2")
                P2 = rw.tile([128, 2, G4], F32R, name="P2")
                Wsrep = rw.tile([128, H], F32, name="Wsrep")
                nc.sync.dma_start(WqT[:], WqT_in[:])
                nc.sync.dma_start(WhhT[:], WhhT_in[:])
                nc.sync.dma_start(Wsrep[:], Wsrep_in[:])
                nc.vector.tensor_scalar_mul(WhhT[:, :, 0:1024],
                                            WhhT[:, :, 0:1024], 0.5)
                nc.vector.tensor_scalar_mul(WhhT[:, :, 1536:2048],
                                            WhhT[:, :, 1536:2048], 0.5)

                _emit_init(nc, tc, tk2, P2, h2aT, cst, ones1, bd,
                           encT_in, embT_in, WkT_in, WihcT_in, WiheT_in,
                           bq_in, bk_in, bih_in, bhh_in, G0_dram)

                with (
                    tc.tile_pool(name="rec_sb", bufs=2) as rsb,
                    tc.tile_pool(name="rec_ps", bufs=1, space="PSUM") as rps,
                    tc.tile_pool(name="g0_sb", bufs=4) as gsb,
                ):
                    for t in range(T):
                        _emit_step(nc, t, rsb, rps, gsb, h2aT, cst, WqT, WhhT,
                                   tk2, P2, Wsrep, E4, E4T, id4, bd, attn_all,
                                   half4, G0_dram)

            nc.sync.dma_start(attn_out[:], attn_all[:])
            nc.sync.dma_start(cT_out[:], cst[:])
            nc.sync.dma_start(hT_out[:], h2aT[:, :, 4 * T:4 * T + 4])

            _emit_projection(nc, tc, h2aT, ones1, WoutT_in, bout_in, logp_out)

    nc.finalize()
    return nc


_NC_CACHE = None


def _get_module():
    global _NC_CACHE
    if _NC_CACHE is None:
        _NC_CACHE = build_module()
    return _NC_CACHE


def _tileT(mat, free):
    """[R, C] -> [128, R//128, C] with out[i, k, :] = mat[128k+i, :]."""
    R = mat.shape[0]
    return np.ascontiguousarray(mat.reshape(R // 128, 128, free).transpose(1, 0, 2))


def _build_in_maps(inputs):
    return _prep_in_maps(**inputs)


def _prep_in_maps(encoder_outputs, enc_h, enc_c, target_tensor, embedding,
                  Wq, bq, Wk, bk, Ws, bs, Wih, bih, Whh, bhh, Wout, bout):
    encoder_outputs = np.asarray(encoder_outputs, np.float32)
    enc_h = np.asarray(enc_h, np.float32)
    enc_c = np.asarray(enc_c, np.float32)
    target_tensor = np.asarray(target_tensor)
    embedding = np.asarray(embedding, np.float32)
    Wq = np.asarray(Wq, np.float32); bq = np.asarray(bq, np.float32)
    Wk = np.asarray(Wk, np.float32); bk = np.asarray(bk, np.float32)
    Ws = np.asarray(Ws, np.float32)
    Wih = np.asarray(Wih, np.float32); bih = np.asarray(bih, np.float32)
    Whh = np.asarray(Whh, np.float32); bhh = np.asarray(bhh, np.float32)
    Wout = np.asarray(Wout, np.float32); bout = np.asarray(bout, np.float32)

    # teacher-forced input tokens and their embeddings (gather on host)
    sos = np.full((B, 1), 1, dtype=target_tensor.dtype)
    tokens = np.concatenate([sos, target_tensor[:, :-1]], axis=1)  # [B, T]
    emb_seq = embedding[tokens]  # [B, T, H]

    WqT = _tileT(Wq.T, H)
    WkT = _tileT(Wk.T, H)
    WhhT = _tileT(Whh.T, G4)
    WihcT = _tileT(Wih[:, H:2 * H].T, G4)
    WiheT = _tileT(Wih[:, 0:H].T, G4)
    Wsrep = np.ascontiguousarray(np.broadcast_to(Ws[0:1, :], (128, H)))
    E4 = np.zeros((4, 128), np.float32)
    for g in range(4):
        E4[g, 32 * g:32 * g + 32] = 1.0
    E4T = np.ascontiguousarray(E4.T)
    ones1 = np.ones((1, 128), np.float32)
    id4 = np.eye(4, dtype=np.float32)
    zeros8 = np.zeros((128, 2, 4), np.float32)
    Wp = np.zeros((H, VP), np.float32)
    Wp[:, :V] = Wout.T
    WoutT = np.ascontiguousarray(
        Wp.reshape(4, 128, NT, 512).transpose(2, 1, 0, 3)).astype(ml_dtypes.bfloat16)
    boutp = np.zeros((1, VP), np.float32)
    boutp[0, :V] = bout

    shared = {
        "WqT_in": WqT, "WkT_in": WkT, "WhhT_in": WhhT, "WihcT_in": WihcT,
        "WiheT_in": WiheT, "bq_in": bq.reshape(1, H), "bk_in": bk.reshape(1, H),
        "bih_in": bih.reshape(1, G4), "bhh_in": bhh.reshape(1, G4),
        "Wsrep_in": Wsrep, "E4_in": E4, "E4T_in": E4T, "ones1_in": ones1,
        "id4_in": id4, "zeros8_in": zeros8, "WoutT_in": WoutT, "bout_in": boutp,
    }

    in_maps = []
    for c in range(NCORES):
        bsl = slice(BL * c, BL * (c + 1))
        h0T = np.ascontiguousarray(
            enc_h[0, bsl, :].T.reshape(4, 128, BL).transpose(1, 0, 2))
        c0 = np.ascontiguousarray(enc_c[0, bsl, :])
        E = encoder_outputs[bsl]  # [4, S, H]
        encT = np.ascontiguousarray(
            E.reshape(BL, 2, 32, 4, 128).transpose(4, 3, 1, 0, 2)
            .reshape(128, 4, 2, 128))
        R = emb_seq[bsl].transpose(1, 0, 2).reshape(T * BL, H)  # rows (t,b)
        embT = np.ascontiguousarray(R.T.reshape(4, 128, 2, 128).transpose(1, 0, 2, 3))
        m = dict(shared)
        m.update({"h0T_in": h0T, "c0_in": c0, "encT_in": encT, "embT_in": embT})
        in_maps.append(m)
    return in_maps


def _assemble_outputs(results):
    log_probs = np.empty((B, T, V), np.float32)
    attentions = np.empty((B, T, S), np.float32)
    hT = np.empty((1, B, H), np.float32)
    cT = np.empty((1, B, H), np.float32)
    for c in range(NCORES):
        out = results[c]
        bsl = slice(BL * c, BL * (c + 1))
        lp = out["logp_out"].reshape(256, VP)[:, :V]
        log_probs[bsl] = lp.reshape(T, BL, V).transpose(1, 0, 2)
        A = out["attn_out"]  # [128, T, 2]
        attentions[bsl] = (
            A.reshape(BL, 32, T, 2).transpose(0, 2, 3, 1).reshape(BL, T, S))
        hT[0, bsl] = out["hT_out"].transpose(2, 1, 0).reshape(BL, H)
        cT[0, bsl] = out["cT_out"]
    return log_probs, (hT, cT), attentions


def kernel(encoder_outputs, enc_h, enc_c, target_tensor, embedding,
           Wq, bq, Wk, bk, Ws, bs, Wih, bih, Whh, bhh, Wout, bout):
    in_maps = _prep_in_maps(encoder_outputs, enc_h, enc_c, target_tensor,
                            embedding, Wq, bq, Wk, bk, Ws, bs, Wih, bih,
                            Whh, bhh, Wout, bout)
    nc = _get_module()
    res = run_bass_kernel_spmd(nc, in_maps, core_ids=list(range(NCORES)))
    return _assemble_outputs(res.results)


if __name__ == "__main__":
    _get_module()
    print("module built ok")
